# revision 1
# baseline (speedup 1.0000x reference)
"""3-layer GAT + MLP head on trn2, node-sharded across 8 NeuronCores.

Strategy (per sharding hint): dst nodes partitioned 8 ways (6250/core).
Per layer: each core computes h|sl|dl for its own node shard (PE matmul),
AllGather replicates the [h|sl] payload to every core, then each core
processes its ~106k incoming edges: indirect-DMA gather of source payload
rows, segment softmax (denominator accumulated in the same matmul), and
scatter-add into per-dst-block PSUM via a one-hot matmul. Edges are
host-grouped by (core, dst-block-of-128) so the one-hot is block-local.
"""
import sys, os, types
sys.path.insert(0, '/opt/trn_rl_repo')
import numpy as np
import concourse.bass as bass
import concourse.bacc as bacc
import concourse.tile as tile
from concourse import mybir
from concourse import bass_utils
from concourse.bass_utils import run_bass_kernel_spmd
from concourse.masks import make_identity

N = 50000
F0 = 128
HID = 64
H = 4
DH = 256          # H*HID
OUTD = 40
NEG = 0.2
NC8 = 8
SH = 6250         # dst nodes per core
NB = 49           # 128-node blocks per core
SHP = NB * 128    # 6272
PAY = 260         # payload width: h(256) + sl(4)
EXT = 264         # transform width: h + sl + dl
SUPER = 10        # edge blocks batched per DVE chain

LAST_EXEC_NS = None


def _install_ntff_hook():
    if "antenv.axon_hooks" in sys.modules:
        return
    try:
        import antenv
        from trn_agent_boot.trn_boot import _ntff_profile_via_ctypes
        hook = _ntff_profile_via_ctypes('/opt/axon/libaxon_pjrt.so')
    except Exception:
        hook = None
    m = types.ModuleType("antenv.axon_hooks")
    m.get_axon_ntff_profile_hook = lambda: hook
    m.set_axon_ntff_profile_hook = lambda h: None
    sys.modules["antenv.axon_hooks"] = m
    bass_utils.upload_artifacts = lambda d: f"local:{d}"


def _prep_edges(edge_index):
    src = np.asarray(edge_index[0], dtype=np.int64)
    dst = np.asarray(edge_index[1], dtype=np.int64)
    loop = np.arange(N, dtype=np.int64)
    src = np.concatenate([src, loop])
    dst = np.concatenate([dst, loop])

    core = dst // SH
    ldst = dst - core * SH
    blk = ldst // 128
    dloc = (ldst - blk * 128).astype(np.float32)
    gsrc = ((src // SH) * SHP + (src % SH)).astype(np.int32)

    key = (core * NB + blk).astype(np.int64)
    order = np.argsort(key, kind='stable')
    key_s = key[order]
    counts = np.bincount(key_s, minlength=NC8 * NB)
    C_blk = int(np.ceil(counts.max() / 128))
    cap = C_blk * 128

    starts = np.zeros(NC8 * NB + 1, dtype=np.int64)
    starts[1:] = np.cumsum(counts)
    pos = np.arange(len(key_s)) - starts[key_s]

    a_gsrc = np.zeros((NC8, NB, cap), dtype=np.int32)
    a_dloc = np.full((NC8, NB, cap), -1.0, dtype=np.float32)
    a_ddst = np.zeros((NC8, NB, cap), dtype=np.int32)
    c_s = (key_s // NB).astype(np.int64)
    b_s = (key_s % NB).astype(np.int64)
    a_gsrc[c_s, b_s, pos] = gsrc[order]
    a_dloc[c_s, b_s, pos] = dloc[order]
    a_ddst[c_s, b_s, pos] = ldst[order].astype(np.int32)

    TOTB = NB * C_blk
    gsrcT = [np.ascontiguousarray(a_gsrc[c].reshape(TOTB, 128).T) for c in range(NC8)]
    dlocT = [np.ascontiguousarray(a_dloc[c].reshape(TOTB, 128).T) for c in range(NC8)]
    ddstT = [np.ascontiguousarray(a_ddst[c].reshape(TOTB, 128).T) for c in range(NC8)]
    return C_blk, gsrcT, dlocT, ddstT


def _pack_attn(a_s, a_d):
    # As_packed[h*64+c, h] = a[h, c]
    p_s = np.zeros((DH, H), dtype=np.float32)
    p_d = np.zeros((DH, H), dtype=np.float32)
    for h in range(H):
        p_s[h * HID:(h + 1) * HID, h] = a_s[h]
        p_d[h * HID:(h + 1) * HID, h] = a_d[h]
    return p_s, p_d


def _build(C_blk):
    TOTB = NB * C_blk
    f32 = mybir.dt.float32
    i32 = mybir.dt.int32
    nc = bacc.Bacc("TRN2", target_bir_lowering=False, debug=False, num_swdge_queues=4)

    xT = nc.dram_tensor("xT", [F0, SHP], f32, kind="ExternalInput")
    W1e = nc.dram_tensor("W1e", [F0, EXT], f32, kind="ExternalInput")
    W2e = nc.dram_tensor("W2e", [DH, EXT], f32, kind="ExternalInput")
    W3e = nc.dram_tensor("W3e", [DH, EXT], f32, kind="ExternalInput")
    Wm1 = nc.dram_tensor("Wm1", [DH, DH], f32, kind="ExternalInput")
    Wm2 = nc.dram_tensor("Wm2", [DH, OUTD], f32, kind="ExternalInput")
    gsrcT = nc.dram_tensor("gsrcT", [128, TOTB], i32, kind="ExternalInput")
    ddstT = nc.dram_tensor("ddstT", [128, TOTB], i32, kind="ExternalInput")
    dlocT = nc.dram_tensor("dlocT", [128, TOTB], f32, kind="ExternalInput")
    iotaF = nc.dram_tensor("iotaF", [128, 128], f32, kind="ExternalInput")
    out = nc.dram_tensor("out", [SHP, OUTD], f32, kind="ExternalOutput")

    pay_stage = nc.dram_tensor("pay_stage", [SHP, PAY], f32)
    dl_tab = nc.dram_tensor("dl_tab", [SHP, 4], f32)
    pay_full = nc.dram_tensor("pay_full", [NC8 * SHP, PAY], f32, addr_space="Shared")

    with tile.TileContext(nc) as tc:
        with tc.tile_pool(name="const", bufs=1) as cp, \
             tc.tile_pool(name="work", bufs=2) as wp, \
             tc.tile_pool(name="zt", bufs=1) as zp, \
             tc.tile_pool(name="psA", bufs=2, space="PSUM") as psA, \
             tc.tile_pool(name="psB", bufs=2, space="PSUM") as psB, \
             tc.tile_pool(name="psT", bufs=2, space="PSUM") as psT, \
             tc.tile_pool(name="psO", bufs=2, space="PSUM") as psO:

            ident = cp.tile([128, 128], f32)
            make_identity(nc, ident[:])
            iota_sb = cp.tile([128, 128], f32)
            nc.sync.dma_start(out=iota_sb[:], in_=iotaF[:])
            gsrc_sb = cp.tile([128, TOTB], i32)
            nc.sync.dma_start(out=gsrc_sb[:], in_=gsrcT[:])
            ddst_sb = cp.tile([128, TOTB], i32)
            nc.sync.dma_start(out=ddst_sb[:], in_=ddstT[:])
            dloc_sb = cp.tile([128, TOTB], f32)
            nc.sync.dma_start(out=dloc_sb[:], in_=dlocT[:])

            w1_sb = cp.tile([128, EXT], f32)
            nc.sync.dma_start(out=w1_sb[:], in_=W1e[:])
            w2_sb = [cp.tile([128, EXT], f32, tag=f"w2_{c}", name=f"w2_{c}") for c in range(2)]
            w3_sb = [cp.tile([128, EXT], f32, tag=f"w3_{c}", name=f"w3_{c}") for c in range(2)]
            wm1_sb = [cp.tile([128, DH], f32, tag=f"wm1_{c}", name=f"wm1_{c}") for c in range(2)]
            wm2_sb = [cp.tile([128, OUTD], f32, tag=f"wm2_{c}", name=f"wm2_{c}") for c in range(2)]
            for c in range(2):
                nc.sync.dma_start(out=w2_sb[c][:], in_=W2e[c*128:(c+1)*128, :])
                nc.sync.dma_start(out=w3_sb[c][:], in_=W3e[c*128:(c+1)*128, :])
                nc.sync.dma_start(out=wm1_sb[c][:], in_=Wm1[c*128:(c+1)*128, :])
                nc.sync.dma_start(out=wm2_sb[c][:], in_=Wm2[c*128:(c+1)*128, :])

            zt_a = [zp.tile([128, SHP], f32, tag=f"zta{c}", name=f"zta{c}") for c in range(2)]
            zt_b = [zp.tile([128, SHP], f32, tag=f"ztb{c}", name=f"ztb{c}") for c in range(2)]

            def transform1():
                for nb in range(NB):
                    lhs = wp.tile([128, 128], f32, tag="tlhs")
                    nc.sync.dma_start(out=lhs[:], in_=xT[:, nb*128:(nb+1)*128])
                    ps = psB.tile([128, EXT], f32, tag="tps")
                    nc.tensor.matmul(out=ps[:], lhsT=lhs[:], rhs=w1_sb[:],
                                     start=True, stop=True)
                    hb = wp.tile([128, EXT], f32, tag="thb")
                    nc.vector.tensor_copy(out=hb[:], in_=ps[:])
                    nc.sync.dma_start(out=pay_stage[nb*128:(nb+1)*128, :],
                                      in_=hb[:, 0:PAY])
                    nc.sync.dma_start(out=dl_tab[nb*128:(nb+1)*128, :],
                                      in_=hb[:, PAY:EXT])

            def transformX(zt, w_sb):
                for nb in range(NB):
                    ps = psB.tile([128, EXT], f32, tag="tps")
                    for c in range(2):
                        nc.tensor.matmul(out=ps[:], lhsT=zt[c][:, nb*128:(nb+1)*128],
                                         rhs=w_sb[c][:], start=(c == 0), stop=(c == 1))
                    hb = wp.tile([128, EXT], f32, tag="thb")
                    nc.vector.tensor_copy(out=hb[:], in_=ps[:])
                    nc.sync.dma_start(out=pay_stage[nb*128:(nb+1)*128, :],
                                      in_=hb[:, 0:PAY])
                    nc.sync.dma_start(out=dl_tab[nb*128:(nb+1)*128, :],
                                      in_=hb[:, PAY:EXT])

            def allgather():
                nc.gpsimd.collective_compute(
                    "AllGather", mybir.AluOpType.bypass,
                    replica_groups=[list(range(NC8))],
                    ins=[pay_stage[:]], outs=[pay_full[:]],
                )

            def edge_phase(zt_out):
                for k in range(NB):
                    acc = psA.tile([128, PAY], f32, tag="acc")
                    eb = 0
                    while eb < C_blk:
                        ebn = min(SUPER, C_blk - eb)
                        pay_w = wp.tile([128, SUPER, PAY], f32, tag="payw", bufs=3)
                        dl_w = wp.tile([128, SUPER, 4], f32, tag="dlw", bufs=3)
                        oh_w = wp.tile([128, SUPER, 128], f32, tag="ohw")
                        em_w = wp.tile([128, SUPER, 4], f32, tag="emw")
                        j0 = k * C_blk + eb
                        for j in range(ebn):
                            nc.gpsimd.indirect_dma_start(
                                out=pay_w[:, j, :], out_offset=None, in_=pay_full[:],
                                in_offset=bass.IndirectOffsetOnAxis(
                                    ap=gsrc_sb[:, j0+j:j0+j+1], axis=0),
                            )
                            nc.gpsimd.indirect_dma_start(
                                out=dl_w[:, j, :], out_offset=None, in_=dl_tab[:],
                                in_offset=bass.IndirectOffsetOnAxis(
                                    ap=ddst_sb[:, j0+j:j0+j+1], axis=0),
                            )
                        # alpha = leakyrelu(sl + dl, 0.2); em = exp(min(alpha, 30))
                        nc.vector.tensor_tensor(
                            out=em_w[:, 0:ebn, :], in0=pay_w[:, 0:ebn, 256:260],
                            in1=dl_w[:, 0:ebn, :], op=mybir.AluOpType.add)
                        al2 = wp.tile([128, SUPER, 4], f32, tag="al2")
                        nc.vector.tensor_scalar_mul(
                            out=al2[:, 0:ebn, :], in0=em_w[:, 0:ebn, :], scalar1=NEG)
                        nc.vector.tensor_tensor(
                            out=em_w[:, 0:ebn, :], in0=em_w[:, 0:ebn, :],
                            in1=al2[:, 0:ebn, :], op=mybir.AluOpType.max)
                        nc.vector.tensor_scalar_min(
                            out=em_w[:, 0:ebn, :], in0=em_w[:, 0:ebn, :], scalar1=30.0)
                        nc.scalar.activation(
                            out=em_w[:, 0:ebn, :], in_=em_w[:, 0:ebn, :],
                            func=mybir.ActivationFunctionType.Exp)
                        # premult h by em per head; write em into cols 256:260
                        for h in range(H):
                            nc.vector.tensor_tensor(
                                out=pay_w[:, 0:ebn, h*HID:(h+1)*HID],
                                in0=pay_w[:, 0:ebn, h*HID:(h+1)*HID],
                                in1=em_w[:, 0:ebn, h:h+1].to_broadcast([128, ebn, HID]),
                                op=mybir.AluOpType.mult)
                        nc.vector.tensor_copy(
                            out=pay_w[:, 0:ebn, 256:260], in_=em_w[:, 0:ebn, :])
                        # one-hot over dst window
                        nc.vector.tensor_tensor(
                            out=oh_w[:, 0:ebn, :],
                            in0=dloc_sb[:, j0:j0+ebn, None].to_broadcast([128, ebn, 128]),
                            in1=iota_sb[:, None, :].to_broadcast([128, ebn, 128]),
                            op=mybir.AluOpType.is_equal)
                        for j in range(ebn):
                            nc.tensor.matmul(
                                out=acc[:], lhsT=oh_w[:, j, :], rhs=pay_w[:, j, :],
                                start=(eb + j == 0), stop=(eb + j == C_blk - 1))
                        eb += ebn
                    # finalize dst block k
                    acc_sb = wp.tile([128, PAY], f32, tag="accsb")
                    nc.vector.tensor_copy(out=acc_sb[:], in_=acc[:])
                    rec = wp.tile([128, 4], f32, tag="rec")
                    nc.vector.reciprocal(out=rec[:], in_=acc_sb[:, 256:260])
                    z = wp.tile([128, DH], f32, tag="z")
                    for h in range(H):
                        nc.vector.tensor_tensor(
                            out=z[:, h*HID:(h+1)*HID], in0=acc_sb[:, h*HID:(h+1)*HID],
                            in1=rec[:, h:h+1].to_broadcast([128, HID]),
                            op=mybir.AluOpType.mult)
                    nc.scalar.activation(out=z[:], in_=z[:],
                                         func=mybir.ActivationFunctionType.Relu)
                    for c in range(2):
                        pt = psT.tile([128, 128], f32, tag="tp")
                        nc.tensor.transpose(out=pt[:], in_=z[:, c*128:(c+1)*128],
                                            identity=ident[:])
                        nc.vector.tensor_copy(out=zt_out[c][:, k*128:(k+1)*128],
                                              in_=pt[:])

            # layer 1
            transform1()
            allgather()
            edge_phase(zt_a)
            # layer 2
            transformX(zt_a, w2_sb)
            allgather()
            edge_phase(zt_b)
            # layer 3
            transformX(zt_b, w3_sb)
            allgather()
            edge_phase(zt_a)
            # MLP head
            for nb in range(NB):
                ps = psB.tile([128, DH], f32, tag="tps")
                for c in range(2):
                    nc.tensor.matmul(out=ps[:], lhsT=zt_a[c][:, nb*128:(nb+1)*128],
                                     rhs=wm1_sb[c][:], start=(c == 0), stop=(c == 1))
                m1 = wp.tile([128, DH], f32, tag="m1")
                nc.scalar.activation(out=m1[:], in_=ps[:],
                                     func=mybir.ActivationFunctionType.Relu)
                m1t = wp.tile([128, 2, 128], f32, tag="m1t")
                for c in range(2):
                    pt = psT.tile([128, 128], f32, tag="tp")
                    nc.tensor.transpose(out=pt[:], in_=m1[:, c*128:(c+1)*128],
                                        identity=ident[:])
                    nc.vector.tensor_copy(out=m1t[:, c, :], in_=pt[:])
                po = psO.tile([128, OUTD], f32, tag="po")
                for c in range(2):
                    nc.tensor.matmul(out=po[:], lhsT=m1t[:, c, :], rhs=wm2_sb[c][:],
                                     start=(c == 0), stop=(c == 1))
                ob = wp.tile([128, OUTD], f32, tag="ob")
                nc.vector.tensor_copy(out=ob[:], in_=po[:])
                nc.sync.dma_start(out=out[nb*128:(nb+1)*128, :], in_=ob[:])
    nc.finalize()
    return nc


def kernel(x, edge_index, W1, as1, ad1, b1, W2, as2, ad2, b2, W3, as3, ad3, b3,
           Wm1, bm1, Wm2, bm2):
    global LAST_EXEC_NS
    _install_ntff_hook()

    x = np.asarray(x, dtype=np.float32)
    C_blk, gsrcT, dlocT, ddstT = _prep_edges(edge_index)

    p1s, p1d = _pack_attn(np.asarray(as1, np.float32), np.asarray(ad1, np.float32))
    p2s, p2d = _pack_attn(np.asarray(as2, np.float32), np.asarray(ad2, np.float32))
    p3s, p3d = _pack_attn(np.asarray(as3, np.float32), np.asarray(ad3, np.float32))
    W1 = np.asarray(W1, np.float32); W2 = np.asarray(W2, np.float32)
    W3 = np.asarray(W3, np.float32)
    W1e = np.concatenate([W1, W1 @ p1s, W1 @ p1d], axis=1)
    W2e = np.concatenate([W2, W2 @ p2s, W2 @ p2d], axis=1)
    W3e = np.concatenate([W3, W3 @ p3s, W3 @ p3d], axis=1)

    iotaF = np.tile(np.arange(128, dtype=np.float32)[None, :], (128, 1))

    in_maps = []
    for c in range(NC8):
        xs = np.zeros((SHP, F0), dtype=np.float32)
        xs[:SH] = x[c*SH:(c+1)*SH]
        in_maps.append({
            "xT": np.ascontiguousarray(xs.T),
            "W1e": W1e, "W2e": W2e, "W3e": W3e,
            "Wm1": np.asarray(Wm1, np.float32), "Wm2": np.asarray(Wm2, np.float32),
            "gsrcT": gsrcT[c], "ddstT": ddstT[c], "dlocT": dlocT[c],
            "iotaF": iotaF,
        })

    nc = _build(C_blk)
    trace = os.environ.get("KERNEL_TRACE", "0") == "1"
    res = run_bass_kernel_spmd(nc, in_maps, list(range(NC8)), trace=trace)
    LAST_EXEC_NS = res.exec_time_ns

    out = np.concatenate([res.results[c]["out"][:SH] for c in range(NC8)], axis=0)
    return out.astype(np.float32)

